# revision 16
# baseline (speedup 1.0000x reference)
"""Trainium2 Bass kernel for nn_MultiHeadAttention_8400956031164.

Full attention block: QKV proj + per-head RMSNorm + RoPE + sliding-window
causal attention (WIN=1024) + output proj.

Sharding: tensor-parallel over heads across 8 cores (2 heads/core), both
batches looped per core. Host sums the 8 partial Wo outputs.

Device-side layout strategy (per core):
  - X^T [D, S] streamed per 512-token group; Q,K produced TRANSPOSED
    [hd=128, s] per head directly from projection (lhsT = W slice).
  - All dense matmuls in fp16 (PSUM accumulation f32): same 1 cyc/col PE
    rate as bf16 but 10 mantissa bits, which drops the base numeric error
    ~8x and buys budget for the fp8 attention core below.
  - PV + softmax denominator run in fp8(e4m3) with
    MatmulPerfMode.DoubleRow over key-tile PAIRS (contraction 256 = 2 fp8
    weights/PE cell, measured 1.96x per-matmul throughput):
      * oacc += DR(v8_pair, p8_pair) with v8 = fp8(v)
      * cacc += DR(vc_pair, p8_pair) where vc columns 0..126 carry the
        fp8 v-quantization RESIDUAL (v - v8) and column 127 carries ones.
        The old denominator matmul's output was 128x redundant (same L
        broadcast to every partition); vc reuses that wasted compute to
        correct v8 back to ~f16 accuracy for free. Row 127 of cacc IS the
        denominator L; a K=1 ones matmul broadcasts it to 128 partitions
        (cost = columns only). Head-dim 127 stays uncorrected: its fp8
        error dilutes over 2048 Wo inputs (~0.1% of output rms).
      * p8 = fp8(exp(SCALE*s - 2)): the -2 bias keeps probs under e4m3's
        240 max (softmax is shift-invariant); SCALE rides the exp() input
        scale so qr/kr stay unit-variance f16. fp8 p is the dominant
        remaining error (~1.2e-2 of the 2e-2 budget) -- concentrated
        (low-entropy) softmax queries see no averaging, which is also why
        fp8 anywhere in the q/k/scores path fails the max-err gate.
  - Scores (K=128, no DR possible) stay f16.
  - RMSNorm in transposed layout: sum(q^2) over hd via all-ones matmul
    (broadcast across partitions in PSUM), 1/sqrt via exp(-0.5*ln) on ACT.
  - RoPE in transposed layout: rotate_half runs on GpSimd (its SW cores
    may read a shifted partition base), with the sign AND the 64-row
    partition roll folded into the host sin tables; keeps the PE free.
  - Scores computed transposed S^T[k, q] (k on partitions) per 128x(<=512)
    block over the sliding window; exp on ACT (PSUM->SBUF, fp8 out into
    p2 pair tiles; invalid pair regions memset on DVE, triangle masks
    applied as fp8 0/1-table multiplies on DVE).
  - Wo: lhsT = normalized out^T slices, accumulate 2 head-chunks, ACT/DVE
    copy PSUM->SBUF (f16), DMA out f16; host sums partials in f64.

Software pipeline (the key to keeping PE busy): per 512-token step N
the emission order is [input-DMA prefetch for N+2] + [proj chains of N,
with the attention blocks of step N-1 interleaved between chains] + [Wo
of step N-2]. The ACT exp stream of step N-1 then overlaps the PE
projection matmuls of step N instead of serializing behind them, and the
PE never waits on the softmax-normalization (DVE) tail. Deep tile pools
(x 6, out 6) decouple both DMA directions from compute: input transfers
start two steps before use, and Wo's PSUM->SBUF copies never wait on the
output-DMA drain. PSUM banks: 3 proj/Wo + 2 scores + 3 attn (oacc/cacc/
L-broadcast).
"""

import functools

import numpy as np
import ml_dtypes
from contextlib import ExitStack

import concourse.bass as bass
import concourse.tile as tile
import concourse.mybir as mybir
import concourse.hw_specs as _hw_specs
from concourse import bacc, bass_utils


def _patch_act_tables():
    """Steer every activation in this kernel (Square/Ln/Exp/Copy) to the one
    ACT table set that really contains them all (natural_log_exp_and_others),
    so the greedy first-containing-set chooser never thrashes table loads.
    Only *removes* candidate sets; chosen ids still match act_info.json."""
    if getattr(_patch_act_tables, "done", False):
        return
    orig = _hw_specs.get_activation_tables
    AFT = mybir.ActivationFunctionType
    drop = {AFT.Exp, AFT.Ln, AFT.Square, AFT.Copy, AFT.Identity}

    @functools.cache
    def patched(module_arch):
        t = {k: set(v) for k, v in orig(module_arch).items()}
        for name, funcs in t.items():
            if name != "natural_log_exp_and_others":
                funcs -= drop
        return t

    _hw_specs.get_activation_tables = patched
    bacc.get_activation_tables = patched
    _patch_act_tables.done = True


_patch_act_tables()

B, S, D, H, HD, WIN = 2, 2048, 2048, 16, 128, 1024
EPS = 1e-6
SCALE = HD ** -0.5
NCORES = 8
HLOC = H // NCORES          # heads per core = 2
NL = HLOC * HD              # local head dims = 256
SG = 512                    # token group size
G = S // SG                 # groups per batch = 4
NDK = D // 128              # contraction chunks = 16
EXP_BIAS = -2.0             # shift inside exp() so probs fit e4m3 (<240)
OCOL = 64                   # vc ones-column -> L lands on psum partition 64
                            # (base-partition-aligned, required by the K=1 bcast
                            # matmul's base-partition constraint)

F32 = mybir.dt.float32
F16 = mybir.dt.float16
F8 = mybir.dt.float8e4
AF = mybir.ActivationFunctionType
DRW = mybir.MatmulPerfMode.DoubleRow

_CACHE = {}


def _build():
    nc = bacc.Bacc(trn_type="TRN2", target_bir_lowering=False, debug=False)

    def din(name, shape, dt):
        return nc.dram_tensor(name, shape, dt, kind="ExternalInput").ap()

    # All inputs are host-pre-tiled to be partition-major contiguous so every
    # DMA is ~128 descriptors of large contiguous runs.
    xt = din("xt", [B * G, 128, NDK * SG], F16)       # per (b,g) [128, 16*512]
    wq = din("wq", [128, NDK * NL], F16)
    wk = din("wk", [128, NDK * NL], F16)
    wv = din("wv", [128, NDK * NL], F16)
    wo = din("wo", [128, HLOC * D], F16)
    cs = din("cs", [B * G, 128, 4 * SG], F16)         # packed cq|sq|ck|sk
    ones_d = din("ones_d", [128, 128], F16)
    tri_d = din("tri_d", [128, 256], F8)     # causal-keep | window-keep 0/1
    opart = nc.dram_tensor("opart", [B * S, D], F16, kind="ExternalOutput").ap()

    steps = [(b, g) for b in range(B) for g in range(G)]
    NSTEP = len(steps)

    with tile.TileContext(nc) as tc, ExitStack() as ctx:
        const = ctx.enter_context(tc.tile_pool(name="const", bufs=1))
        wpool = ctx.enter_context(tc.tile_pool(name="w", bufs=1))
        xpool = ctx.enter_context(tc.tile_pool(name="x", bufs=6))
        cspool = ctx.enter_context(tc.tile_pool(name="cs", bufs=3))
        qpool = ctx.enter_context(tc.tile_pool(name="qr", bufs=5))
        kpool = ctx.enter_context(tc.tile_pool(name="kr", bufs=10))
        vpool = ctx.enter_context(tc.tile_pool(name="v", bufs=10))
        vcpool = ctx.enter_context(tc.tile_pool(name="vc", bufs=10))
        rpool = ctx.enter_context(tc.tile_pool(name="rms", bufs=2))
        ppool = ctx.enter_context(tc.tile_pool(name="p", bufs=5))
        lpool = ctx.enter_context(tc.tile_pool(name="lin", bufs=4))
        opool = ctx.enter_context(tc.tile_pool(name="osb", bufs=6))
        outp = ctx.enter_context(tc.tile_pool(name="out", bufs=6))
        psA = ctx.enter_context(tc.tile_pool(name="psA", bufs=3, space="PSUM"))
        psS = ctx.enter_context(tc.tile_pool(name="psS", bufs=2, space="PSUM"))
        psO = ctx.enter_context(tc.tile_pool(name="psO", bufs=3, space="PSUM"))

        inputs = {}    # idx -> (xh0, xh1, cst)
        qrs = {}       # idx -> {h: [128,SG] f16}
        KrT = {}       # (b, h, g) -> [128,SG] f16
        Vt2 = {}       # (b, token-tile pair) -> [128,2,NL] fp8 (v8)
        Vc2 = {}       # (b, token-tile pair) -> [128,2,NL] fp8 (v-v8 | ones)
        osbs_all = {}  # idx -> {h: [128,SG] f16}

        def issue_dmas(idx):
            b, g = steps[idx]
            bg = b * G + g
            src = xt[bg].rearrange("p (a s) -> p a s", s=SG)
            xh0 = xpool.tile([128, 8, SG], F16, tag="xt", name="xh0")
            xh1 = xpool.tile([128, 8, SG], F16, tag="xt", name="xh1")
            nc.sync.dma_start(xh0[:], src[:, 0:8, :])
            nc.sync.dma_start(xh1[:], src[:, 8:16, :])
            cst = cspool.tile([128, 4, SG], F16, tag="cs")
            nc.sync.dma_start(
                cst[:], cs[bg].rearrange("p (f s) -> p f s", s=SG))
            inputs[idx] = (xh0, xh1, cst)

        def attn_thunks(idx):
            """Attention micro-ops (PE-centric) for step idx, as a list of
            thunks to interleave between the next step's projection chains."""
            b, g = steps[idx]
            qr_tiles = qrs[idx]
            thunks = []
            osbs = {}
            osbs_all[idx] = osbs

            def make_head(h):
                qr_t = qr_tiles[h]
                kt0s = list(range(max(0, 4 * g - 8), 4 * g + 4, 2))
                npair = len(kt0s)
                state = {}
                pend = []
                LAGP = 2

                def start():
                    state["oacc"] = psO.tile([128, SG], F32, tag="o",
                                             name="oacc")
                    state["cacc"] = psO.tile([128, SG], F32, tag="o",
                                             name="cacc")

                def emit_pv(j, first, last):
                    qoff, n, p2, kt0 = pend[j]
                    v8 = Vt2[(b, kt0 // 2)][:, :, h * HD:(h + 1) * HD]
                    vc = Vc2[(b, kt0 // 2)][:, :, h * HD:(h + 1) * HD]
                    nc.tensor.matmul(
                        state["oacc"][:, qoff:qoff + n], v8, p2[:],
                        start=first, stop=last, perf_mode=DRW)
                    nc.tensor.matmul(
                        state["cacc"][:, qoff:qoff + n], vc, p2[:],
                        start=first, stop=last, perf_mode=DRW)

                def pairblock(i):
                    kt0 = kt0s[i]
                    qt_lo = max(4 * g, kt0)
                    qt_hi = min(4 * g + 3, kt0 + 9)
                    qoff = 128 * (qt_lo - 4 * g)
                    n = 128 * (qt_hi - qt_lo + 1)
                    p2 = ppool.tile([128, 2, n], F8, tag="p")
                    for sl, kt in enumerate((kt0, kt0 + 1)):
                        a_lo = max(4 * g, kt)
                        a_hi = min(4 * g + 3, kt + 8)
                        koff = 128 * (a_lo - 4 * g)
                        kn = 128 * (a_hi - a_lo + 1)
                        sc = psS.tile([128, kn], F32, tag="score")
                        kr_t = KrT[(b, h, kt // 4)]
                        c = (kt % 4) * 128
                        nc.tensor.matmul(sc[:], kr_t[:, c:c + 128],
                                         qr_t[:, koff:koff + kn],
                                         start=True, stop=True)
                        rel = koff - qoff
                        nc.scalar.activation(p2[:, sl, rel:rel + kn], sc[:],
                                             AF.Exp, scale=SCALE,
                                             bias=eps_t[:, 1:2])
                        if rel > 0:
                            nc.vector.memset(p2[:, sl, 0:rel], 0.0)
                        if rel + kn < n:
                            nc.vector.memset(p2[:, sl, rel + kn:n], 0.0)
                        if kt >= 4 * g:
                            # causal triangle: keep kk <= qq (fp8 0/1 mul)
                            nc.vector.tensor_mul(p2[:, sl, rel:rel + 128],
                                                 p2[:, sl, rel:rel + 128],
                                                 tri_t[:, 0:128])
                        if kt + 8 <= 4 * g + 3:
                            # window edge: keep kk >= qq
                            nc.vector.tensor_mul(
                                p2[:, sl, rel + kn - 128:rel + kn],
                                p2[:, sl, rel + kn - 128:rel + kn],
                                tri_t[:, 128:256])
                    pend.append((qoff, n, p2, kt0))
                    if i >= LAGP:
                        emit_pv(i - LAGP, first=(i - LAGP == 0), last=False)

                def tail():
                    for j in range(max(0, npair - LAGP), npair):
                        emit_pv(j, first=(j == 0), last=(j == npair - 1))
                    # cacc -> SBUF f16 (DVE can read only ONE PSUM operand
                    # per op, and we need oacc+cacc sums below). Row OCOL of
                    # csb is L; broadcast it to all 128 partitions with a
                    # K=1 ones matmul (cols-bound).
                    csb = lpool.tile([128, SG], F16, tag="lsb")
                    nc.vector.tensor_copy(csb[:], state["cacc"][:])
                    lbc = psO.tile([128, SG], F32, tag="o", name="lbc")
                    nc.tensor.matmul(lbc[:], ones_t[OCOL:OCOL + 1, :],
                                     csb[OCOL:OCOL + 1, :],
                                     start=True, stop=True)
                    linv = lpool.tile([128, SG], F32, tag="lin")
                    nc.vector.reciprocal_approx_fast(linv[:], lbc[:])
                    # Full-partition ops only (DVE partition slices must be
                    # 32-aligned). Row OCOL computes (oacc+L)/L = true+1
                    # exactly; the resulting constant Wo-row vector is
                    # subtracted on the HOST (see kernel()).
                    osb = opool.tile([128, SG], F16, tag="osb")
                    nc.vector.tensor_add(osb[:], state["oacc"][:], csb[:])
                    nc.vector.tensor_mul(osb[:], osb[:], linv[:])
                    osbs[h] = osb

                thunks.append(start)
                for i in range(npair):
                    thunks.append(lambda i=i: pairblock(i))
                thunks.append(tail)

            for h in range(HLOC):
                make_head(h)
            return thunks

        def wo_thunks(idx):
            """Wo micro-ops for step idx as thunks: one per (st, dg) psum
            pair, plus the output DMA after each st row completes."""
            b, g = steps[idx]
            s0 = b * S + g * SG
            osbs_ = osbs_all.pop(idx)
            thunks = []
            state = {}

            def pair(st, dg):
                if dg == 0:
                    state["ot"] = outp.tile([128, D], F16, tag="out",
                                            name="ot")
                pso = psA.tile([128, 512], F32, tag="a")
                nc.tensor.matmul(pso[:],
                                 osbs_[0][:, st * 128:(st + 1) * 128],
                                 wo_t[:, 0, dg * 512:(dg + 1) * 512],
                                 start=True, stop=False)
                nc.tensor.matmul(pso[:],
                                 osbs_[1][:, st * 128:(st + 1) * 128],
                                 wo_t[:, 1, dg * 512:(dg + 1) * 512],
                                 start=False, stop=True)
                dst = state["ot"][:, dg * 512:(dg + 1) * 512]
                if dg % 2 == 0:
                    nc.scalar.copy(dst, pso[:])
                else:
                    nc.vector.tensor_copy(dst, pso[:])
                if dg == 3:
                    row = s0 + st * 128
                    nc.sync.dma_start(opart[row:row + 128, :],
                                      state["ot"][:])

            for st in range(4):
                for dg in range(4):
                    thunks.append(lambda st=st, dg=dg: pair(st, dg))
            return thunks

        # DMA issue order = first-use order, split into small tiles so the
        # PE's first projection matmuls start as soon as the first x / wq
        # chunks land (deps are tile-granular). Startup chunks ride
        # different engines' issue queues for parallel DMA; wo_t is not
        # needed until step 2.
        wqsrc = wq.rearrange("p (a n) -> p a n", n=NL)
        wq_t = wpool.tile([128, NDK, NL], F16, tag="wq")
        nc.sync.dma_start(wq_t[:, 0:6, :], wqsrc[:, 0:6, :])
        nc.gpsimd.dma_start(wq_t[:, 6:11, :], wqsrc[:, 6:11, :])
        nc.scalar.dma_start(wq_t[:, 11:16, :], wqsrc[:, 11:16, :])
        src0 = xt[0].rearrange("p (a s) -> p a s", s=SG)
        xh0_0 = xpool.tile([128, 8, SG], F16, tag="xt", name="xh0")
        nc.sync.dma_start(xh0_0[:, 0:3, :], src0[:, 0:3, :])
        nc.gpsimd.dma_start(xh0_0[:, 3:5, :], src0[:, 3:5, :])
        nc.scalar.dma_start(xh0_0[:, 5:8, :], src0[:, 5:8, :])
        xh1_0 = xpool.tile([128, 8, SG], F16, tag="xt", name="xh1")
        nc.gpsimd.dma_start(xh1_0[:], src0[:, 8:16, :])
        cst_0 = cspool.tile([128, 4, SG], F16, tag="cs")
        nc.sync.dma_start(cst_0[:], cs[0].rearrange("p (f s) -> p f s", s=SG))
        inputs[0] = (xh0_0, xh1_0, cst_0)
        wk_t = wpool.tile([128, NDK, NL], F16, tag="wk")
        nc.sync.dma_start(wk_t[:], wk.rearrange("p (a n) -> p a n", n=NL))
        ones_t = const.tile([128, 128], F16, tag="ones")
        nc.sync.dma_start(ones_t[:], ones_d)
        one8_t = const.tile([128, 1], F8, tag="one8")
        nc.vector.tensor_copy(one8_t[:], ones_t[:, 0:1])
        eps_t = const.tile([128, 2], F32, tag="eps")
        nc.vector.memset(eps_t[:, 0:1], EPS)
        nc.vector.memset(eps_t[:, 1:2], EXP_BIAS)
        wv_t = wpool.tile([128, NDK, NL], F16, tag="wv")
        nc.sync.dma_start(wv_t[:], wv.rearrange("p (a n) -> p a n", n=NL))
        tri_t = const.tile([128, 256], F8, tag="tri")
        nc.sync.dma_start(tri_t[:], tri_d)
        issue_dmas(1)
        wo_t = wpool.tile([128, HLOC, D], F16, tag="wo")
        nc.sync.dma_start(wo_t[:], wo.rearrange("p (c d) -> p c d", d=D))
        issue_dmas(2)
        for idx in range(NSTEP):
            b, g = steps[idx]
            if 1 <= idx and idx + 2 < NSTEP:
                issue_dmas(idx + 2)
            xh0, xh1, cst = inputs.pop(idx)
            cqt = cst[:, 0, :]
            sqt = cst[:, 1, :]
            ckt = cst[:, 2, :]
            skt = cst[:, 3, :]

            def xs(dk):
                t = xh0 if dk < 8 else xh1
                return t[:, dk % 8, :]

            # attention thunks of the previous step, spread over this step's
            # projection chains
            th = attn_thunks(idx - 1) if idx > 0 else []
            tpos = 0

            def run_thunks(target):
                nonlocal tpos
                while tpos < min(target, len(th)):
                    th[tpos]()
                    tpos += 1

            # ---- Q/K transposed projections + RMSNorm + RoPE ----
            # PE pipelining: after each projection chain, emit the previous
            # chain's sum-of-squares matmul and the chain before that's
            # rotate matmul, so PE never waits on ACT/DVE.
            qr_tiles = {}
            qrs[idx] = qr_tiles
            states = []

            def emit_ss(stt):
                ssps = psS.tile([128, SG], F32, tag="score")
                nc.tensor.matmul(ssps[:], ones_t[:], stt["qsq"][:],
                                 start=True, stop=True)
                # 1/sqrt(v) = exp(-0.5*ln(v)) keeps every ACT func in the
                # natural_log_exp_and_others table set (no table thrash).
                rstd = rpool.tile([128, SG], F32, tag="rstd")
                nc.scalar.activation(rstd[:], ssps[:], AF.Ln,
                                     bias=eps_t[:, 0:1], scale=1.0 / HD)
                nc.scalar.activation(rstd[:], rstd[:], AF.Exp, scale=-0.5)
                qn = rpool.tile([128, SG], F16, tag="qn")
                nc.vector.tensor_mul(qn[:], stt["ps"][:], rstd[:])
                t1 = rpool.tile([128, SG], F16, tag="t1")
                cost = cqt if stt["t"] == "q" else ckt
                nc.vector.tensor_mul(t1[:], qn[:], cost[:])
                stt["qn"] = qn
                stt["t1"] = t1

            def emit_rot(stt):
                # rotate_half on GpSimd (SW cores may read a different
                # partition base than they write, as long as both INPUTS
                # share a base): dst[p] = qn[(p+64)%128] * sin_signed[p].
                # The sin tables are partition-rolled by 64 and sign-folded
                # on the host so both inputs align at the same base.
                sint = sqt if stt["t"] == "q" else skt
                dst = stt["dst"]
                qn = stt["qn"]
                nc.gpsimd.tensor_mul(dst[0:64, :], qn[64:128, :],
                                     sint[64:128, :])
                nc.gpsimd.tensor_mul(dst[64:128, :], qn[0:64, :],
                                     sint[0:64, :])
                nc.vector.tensor_add(dst[:], dst[:], stt["t1"][:])

            chains = [("q", 0), ("k", 0), ("q", 1), ("k", 1),
                      ("v", 0), ("v", 1), ("v", 2), ("v", 3)]
            for i, (t, h) in enumerate(chains):
                if t in ("q", "k"):
                    w_t = wq_t if t == "q" else wk_t
                    ps = psA.tile([128, SG], F32, tag="a")
                    for dk in range(NDK):
                        nc.tensor.matmul(
                            ps[:], w_t[:, dk, h * HD:(h + 1) * HD], xs(dk),
                            start=(dk == 0), stop=(dk == NDK - 1))
                    qsq = rpool.tile([128, SG], F16, tag="qsq")
                    nc.scalar.activation(qsq[:], ps[:], AF.Square)
                    if t == "q":
                        dst = qpool.tile([128, SG], F16, tag="qr")
                        qr_tiles[h] = dst
                    else:
                        dst = kpool.tile([128, SG], F16, tag="kr")
                        KrT[(b, h, g)] = dst
                    states.append({"ps": ps, "qsq": qsq, "t": t, "dst": dst})
                else:
                    st = h
                    psv = psA.tile([128, NL], F32, tag="a")
                    for dk in range(NDK):
                        nc.tensor.matmul(
                            psv[:], xs(dk)[:, st * 128:(st + 1) * 128],
                            wv_t[:, dk, :],
                            start=(dk == 0), stop=(dk == NDK - 1))
                    pair_key = (b, (4 * g + st) // 2)
                    if st % 2 == 0:
                        vt2 = vpool.tile([128, 2, NL], F8, tag="v")
                        vc2 = vcpool.tile([128, 2, NL], F8, tag="vc")
                        Vt2[pair_key] = vt2
                        Vc2[pair_key] = vc2
                    else:
                        vt2 = Vt2[pair_key]
                        vc2 = Vc2[pair_key]
                    sl = st % 2
                    nc.vector.tensor_copy(vt2[:, sl, :], psv[:])
                    # vc = v - v8 (fp8 residual); col 127 per head := 1.0
                    # (the ones column that turns cacc row 127 into L)
                    nc.vector.tensor_sub(vc2[:, sl, :], psv[:], vt2[:, sl, :])
                    for hh in range(HLOC):
                        nc.vector.tensor_copy(
                            vc2[:, sl, hh * HD + OCOL:hh * HD + OCOL + 1],
                            one8_t[:, 0:1])
                if 0 <= i - 1 < 4:
                    emit_ss(states[i - 1])
                if 0 <= i - 2 < 4:
                    emit_rot(states[i - 2])
                run_thunks((len(th) * (i + 1)) // len(chains))

            run_thunks(len(th))

            # Wo of step idx-2 (its attention completed during step idx-1)
            if idx - 2 >= 0:
                for t in wo_thunks(idx - 2):
                    t()

        # drain: attention of the last step, then the last two Wo blocks
        for t in attn_thunks(NSTEP - 1) + wo_thunks(NSTEP - 2):
            t()
        for t in wo_thunks(NSTEP - 1):
            t()

    nc.compile()
    return nc


def _host_prep(hidden_states, cos, sin, Wq, Wk, Wv, Wo, q_scale, k_scale):
    f32 = np.float32
    f16 = np.float16
    fp8 = ml_dtypes.float8_e4m3
    hs = np.asarray(hidden_states, f32)
    cos = np.asarray(cos, f32)
    sin = np.asarray(sin, f32)
    qs = np.asarray(q_scale, f32)
    ks = np.asarray(k_scale, f32)

    def ptile(a2d, width, dt):
        """[128*K, W] -> [128, K*W] partition-major contiguous pre-tiling."""
        k = a2d.shape[0] // 128
        return np.ascontiguousarray(
            a2d.reshape(k, 128, width).transpose(1, 0, 2).reshape(128, -1)
        ).astype(dt)

    # xt: per (b,g) block of X^T, pre-tiled
    xt = np.stack([
        ptile(hs[b].T[:, g * SG:(g + 1) * SG], SG, f16)
        for b in range(B) for g in range(G)
    ])   # [B*G, 128, 16*SG]

    qs_rot = np.roll(qs, -64)
    ks_rot = np.roll(ks, -64)
    # rotate_half's -1 on the first half is folded into the sin tables,
    # which are then partition-rolled by 64 so the GpSimd rotate muls read
    # both inputs (qn, sin) at the same partition base. SCALE is NOT folded
    # here -- it rides the exp() input scale on device.
    sgn = np.ones((HD, 1), f32)
    sgn[:64] = -1.0
    cq_full = [(cos[b] * qs[None, :]).T for b in range(B)]            # [HD,S]
    sq_full = [np.roll((sin[b] * qs_rot[None, :]).T * sgn, 64, axis=0)
               for b in range(B)]
    ck_full = [(cos[b] * ks[None, :]).T for b in range(B)]
    sk_full = [np.roll((sin[b] * ks_rot[None, :]).T * sgn, 64, axis=0)
               for b in range(B)]
    cs_all = np.stack([
        np.concatenate([t[:, g * SG:(g + 1) * SG]
                        for t in (cq_full[b], sq_full[b],
                                  ck_full[b], sk_full[b])], axis=1)
        for b in range(B) for g in range(G)
    ]).astype(f16)   # [B*G, 128, 4*SG]
    cs_all = np.ascontiguousarray(cs_all)

    ones = np.ones((128, 128), f16)
    kk = np.arange(128)[:, None]
    qq = np.arange(128)[None, :]
    tri = np.concatenate([(qq >= kk).astype(f32),     # causal keep
                          (kk >= qq).astype(f32)],    # window-edge keep
                         axis=1).astype(fp8)
    shared = {"xt": xt, "cs": cs_all, "ones_d": ones, "tri_d": tri}
    Wq = np.asarray(Wq, f32)
    Wk = np.asarray(Wk, f32)
    Wv = np.asarray(Wv, f32)
    Wo = np.asarray(Wo, f32)
    in_maps = []
    for c in range(NCORES):
        m = dict(shared)
        m["wq"] = ptile(Wq[:, c * NL:(c + 1) * NL], NL, f16)
        m["wk"] = ptile(Wk[:, c * NL:(c + 1) * NL], NL, f16)
        m["wv"] = ptile(Wv[:, c * NL:(c + 1) * NL], NL, f16)
        m["wo"] = ptile(Wo[c * NL:(c + 1) * NL, :], D, f16)
        in_maps.append(m)
    return in_maps


def get_nc():
    if "nc" not in _CACHE:
        _CACHE["nc"] = _build()
    return _CACHE["nc"]


def kernel(hidden_states, cos, sin, Wq, Wk, Wv, Wo, q_scale, k_scale):
    nc = get_nc()
    in_maps = _host_prep(hidden_states, cos, sin, Wq, Wk, Wv, Wo,
                         q_scale, k_scale)
    res = bass_utils.run_bass_kernel_spmd(nc, in_maps,
                                          core_ids=list(range(NCORES)))
    acc = np.zeros((B * S, D), np.float64)
    for r in res.results:
        acc += r["opart"].astype(np.float64)
    # Undo the +1 that the vc ones-column injects into osb row OCOL of every
    # head (the L-broadcast trick): a constant vector per token.
    Wo16 = np.asarray(Wo, np.float32).astype(np.float16).astype(np.float64)
    acc -= Wo16[OCOL::HD, :].sum(axis=0)[None, :]
    return np.ascontiguousarray(
        acc.reshape(B, S, D).astype(np.float32))
